# revision 2
# baseline (speedup 1.0000x reference)
"""Multi-head causal attention (B=4, T=2048, C=768, H=12, HS=64) on 8 trn2 cores.

v4 over the v2 baseline:
  - inputs host-packed partition-major so the whole weight set lands in 5
    large DMAs (was 37 small ones) and x in 12 half-band DMAs ordered so the
    first QK chunk waits only for the first halves; per-descriptor queue
    overhead drops ~5x.
  - diagonal causal masking via DVE multiply with a precomputed triangular
    mask (was gpsimd affine_select on the exp->PV critical chain).
  - output bias added on device (fused into the proj psum->sbuf copy on
    DVE; even cores get the real bias, odd cores zeros).
  - ysb pool deepened (3->6) so psum-recycle semaphores never chain tail
    projections behind recent y DMAs.
  - tail: the last chunk's normalization runs immediately and multiplies
    straight out of the otu psum.

Sharding: 48 (batch, head) units -> 6 per core. Core c: batch c//2, heads
6*(c%2) .. 6*(c%2)+6. Each core computes a partial output projection
y_partial[T, C]; host sums the two partials per batch.
"""

import numpy as np
import ml_dtypes

import concourse.bacc as bacc
import concourse.bass as bass
import concourse.tile as tile
from concourse import mybir
from concourse import bass_utils

B, T, C = 4, 2048, 768
H, HS = 12, 64
HL = 6            # heads per core
NCT = C // 128    # 6 contraction tiles
NTT = T // 128    # 16 t tiles
NTC = T // 512    # 4 t chunks
SCALE = 1.0 / 8.0  # 1/sqrt(HS)

F32 = mybir.dt.float32
BF16 = mybir.dt.bfloat16


def build_kernel(nc, repeat=1):
    xT = nc.dram_tensor("xT", [C, T], BF16, kind="ExternalInput").ap()
    # host-packed partition-major weights, flat per-partition rows so
    # every weight DMA lowers to a plain 2D descriptor
    wqk0 = nc.dram_tensor("wqk0", [128, 2 * NCT * 128], BF16,
                          kind="ExternalInput").ap()
    wqk12 = nc.dram_tensor("wqk12", [128, 2 * 2 * NCT * 128], BF16,
                           kind="ExternalInput").ap()
    wv = nc.dram_tensor("wv", [128, NCT * HL * HS], BF16,
                        kind="ExternalInput").ap()
    wpt = nc.dram_tensor("wpt", [128, 3 * C], BF16,
                         kind="ExternalInput").ap()
    bpb = nc.dram_tensor("bpb", [1, C], F32, kind="ExternalInput").ap()
    y = nc.dram_tensor("y", [T, C], BF16, kind="ExternalOutput").ap()

    with tile.TileContext(nc) as tc:
        with (
            tc.tile_pool(name="consts", bufs=1) as consts,
            tc.tile_pool(name="xw", bufs=1) as xw,
            tc.tile_pool(name="pt", bufs=16) as ptp,
            tc.tile_pool(name="small", bufs=4) as small,
            tc.tile_pool(name="ysb", bufs=6) as ysbp,
            # PSUM budget (8 banks): st 2x2 + otu 1x2 + tt 2x1
            tc.tile_pool(name="ps_st", bufs=2, space="PSUM") as ps_st,
            tc.tile_pool(name="ps_otu", bufs=1, space="PSUM") as ps_otu,
            tc.tile_pool(name="ps_t", bufs=2, space="PSUM") as ps_t,
        ):
            # ---------------- input DMAs ----------------
            # scalar queue: pair-0 QK weights (one DMA), then odd x halves.
            # sync queue: even x halves, then V/proj weights and the rest.
            # First-QK needs wqk[:, 0] + the h0 halves of every band.
            wqk0_sb = xw.tile([128, 2, NCT, 128], BF16, tag="wqk0",
                              name="wqk0")
            nc.scalar.dma_start(
                out=wqk0_sb.rearrange("p a b c -> p (a b c)"), in_=wqk0)
            xt = [xw.tile([128, T], BF16, tag=f"xt{ci}", name=f"xt{ci}")
                  for ci in range(NCT)]
            for h in range(2):
                for ci in range(NCT):
                    q = nc.sync if ci % 2 == 0 else nc.scalar
                    q.dma_start(
                        out=xt[ci][:, h * 1024:(h + 1) * 1024],
                        in_=xT[ci * 128:(ci + 1) * 128,
                               h * 1024:(h + 1) * 1024],
                    )
            wv_sb = xw.tile([128, NCT, HL * HS], BF16, tag="wv", name="wv")
            nc.sync.dma_start(
                out=wv_sb.rearrange("p a b -> p (a b)"), in_=wv)
            wqk12_sb = xw.tile([128, 2, 2, NCT, 128], BF16, tag="wqk12",
                               name="wqk12")
            nc.sync.dma_start(
                out=wqk12_sb.rearrange("p a b c d -> p (a b c d)"),
                in_=wqk12)
            wpt_sb = consts.tile([128, 3, C], BF16, tag="wpt", name="wpt")
            nc.sync.dma_start(
                out=wpt_sb.rearrange("p a b -> p (a b)"), in_=wpt)
            bprow = consts.tile([1, C], F32, tag="bprow", name="bprow")
            nc.scalar.dma_start(out=bprow, in_=bpb)

            # persistent tensors; augmented V column 0 carries the ones row
            # so the softmax denominator lands at psum partition 0.
            vaug = consts.tile([128, NTT, HL, HS + 1], BF16)
            nc.gpsimd.memset(vaug[:, :, :, HS:HS + 1], 1.0)
            pairQ = [consts.tile([128, T], BF16, tag=f"pq{p}", name=f"pq{p}")
                     for p in range(3)]
            pairK = [consts.tile([128, T], BF16, tag=f"pk{p}", name=f"pk{p}")
                     for p in range(3)]
            otn = consts.tile([128, 3, T], BF16)
            ones_rows = consts.tile([128, HS + 1], F32)
            nc.gpsimd.memset(ones_rows, 1.0)
            # upper-triangular (keep c >= r) mask for the diagonal subtiles
            trimask = consts.tile([128, 2, 128], BF16)
            nc.gpsimd.memset(trimask, 1.0)
            nc.gpsimd.affine_select(
                out=trimask, in_=trimask,
                compare_op=mybir.AluOpType.is_ge,
                fill=0.0, base=0,
                pattern=[[0, 2], [1, 128]],
                channel_multiplier=-1,
            )
            # bias broadcast to all 128 partitions (off the critical path)
            bias128 = consts.tile([128, C], F32)
            nc.gpsimd.partition_broadcast(bias128, bprow, channels=128)

            import contextlib
            rep_ctx = (
                tc.For_i(0, repeat, 1,
                         hint_engines=(mybir.EngineType.PE,
                                       mybir.EngineType.DVE,
                                       mybir.EngineType.Activation,
                                       mybir.EngineType.SP,
                                       mybir.EngineType.Pool))
                if repeat > 1 else contextlib.nullcontext()
            )
            with rep_ctx:
                build_phases(nc, tc, consts, xw, ptp, small, ysbp,
                             ps_st, ps_otu, ps_t,
                             xt, wqk0_sb, wqk12_sb, wv_sb, wpt_sb,
                             vaug, pairQ, pairK, otn, ones_rows,
                             trimask, bias128, y)

    nc.compile()
    return nc


def build_phases(nc, tc, consts, xw, ptp, small, ysbp,
                 ps_st, ps_otu, ps_t,
                 xt, wqk0_sb, wqk12_sb, wv_sb, wpt_sb,
                 vaug, pairQ, pairK, otn, ones_rows, trimask, bias128, y):
    def wqk_ap(p, which, ci):
        if p == 0:
            return wqk0_sb[:, which, ci, :]
        return wqk12_sb[:, p - 1, which, ci, :]

    # ---- filler unit builders (each issues one psum-group of PE work) ----
    def qk_unit(p, which, m):
        """Project one 512-col chunk of Q (which=0) or K (which=1) for pair p
        straight into pairQ/pairK (partition-aligned, no staging DMA)."""
        dst = pairQ[p] if which == 0 else pairK[p]
        sl = slice(m * 512, (m + 1) * 512)
        ps = ps_t.tile([128, 512], F32, tag="tt", name=f"qk{p}_{which}_{m}")
        for ci in range(NCT):
            nc.tensor.matmul(ps, wqk_ap(p, which, ci), xt[ci][:, sl],
                             start=(ci == 0), stop=(ci == NCT - 1))
        nc.vector.tensor_copy(out=dst[:, sl], in_=ps)

    def v_unit(tt):
        ps = ps_t.tile([128, HL * HS], F32, tag="tt", name=f"psv{tt}")
        for ci in range(NCT):
            nc.tensor.matmul(ps, xt[ci][:, tt * 128:(tt + 1) * 128],
                             wv_sb[:, ci, :],
                             start=(ci == 0), stop=(ci == NCT - 1))
        nc.vector.tensor_copy(
            out=vaug[:, tt, :, 0:HS],
            in_=ps.rearrange("p (h d) -> p h d", h=HL),
        )

    def norm_rb(state):
        """Row-broadcast of the reciprocal rows on GPSIMD.  The ucode reads
        literal partition 0, so hop the row down first with a tiny DMA."""
        p, m, otu_sb, otu_ps, rbs = state
        for e in range(2):
            stg = small.tile([1, 512], F32, tag="rstg", name=f"rs{p}_{m}_{e}")
            nc.sync.dma_start(out=stg, in_=otu_sb[HS:HS + 1, e, :])
            rb = small.tile([HS, 512], F32, tag="rbb", name=f"rb{p}_{m}_{e}")
            nc.gpsimd.partition_broadcast(rb, stg, channels=HS)
            rbs.append(rb)

    def norm_mul(state, from_psum=False):
        p, m, otu_sb, otu_ps, rbs = state
        for e in range(2):
            otnorm = small.tile([HS, 512], BF16, tag="otnorm", name="otnorm")
            body = (otu_ps if from_psum else otu_sb)[0:HS, e, :]
            nc.vector.tensor_mul(out=otnorm, in0=body, in1=rbs[e])
            nc.sync.dma_start(
                out=otn[64 * e:64 * e + HS, p, m * 512:(m + 1) * 512],
                in_=otnorm,
            )

    def proj_unit(tt, tail=False):
        y1 = ps_t.tile([128, 512], F32, tag="tt", name=f"y1_{tt}")
        y2 = ps_t.tile([128, 256], F32, tag="tt", name=f"y2_{tt}")
        for g in range(3):
            lhs = otn[:, g, tt * 128:(tt + 1) * 128]
            nc.tensor.matmul(y1, lhs, wpt_sb[:, g, 0:512],
                             start=(g == 0), stop=(g == 2))
            nc.tensor.matmul(y2, lhs, wpt_sb[:, g, 512:768],
                             start=(g == 0), stop=(g == 2))
        ysb = ysbp.tile([128, C], BF16, tag="ysb", name="ysb")
        # psum->sbuf copy fused with the device-side bias add; half-granular
        # so the store overlaps the second half.
        nc.vector.tensor_add(out=ysb[:, 0:512], in0=y1, in1=bias128[:, 0:512])
        nc.sync.dma_start(out=y[tt * 128:(tt + 1) * 128, 0:512],
                          in_=ysb[:, 0:512])
        nc.vector.tensor_add(out=ysb[:, 512:768], in0=y2,
                             in1=bias128[:, 512:768])
        q = nc.scalar if tail else nc.sync
        q.dma_start(out=y[tt * 128:(tt + 1) * 128, 512:768],
                    in_=ysb[:, 512:768])

    # ---- static filler schedule ----
    fillers = {
        (0, 0): [lambda tt=t: v_unit(tt) for t in range(0, 4)]
                + [lambda: qk_unit(0, 0, 1), lambda: qk_unit(0, 1, 1)],
        (0, 1): [lambda tt=t: v_unit(tt) for t in range(4, 8)]
                + [lambda: qk_unit(0, 0, 2), lambda: qk_unit(0, 1, 2)],
        (0, 2): [lambda tt=t: v_unit(tt) for t in range(8, 12)]
                + [lambda: qk_unit(0, 0, 3), lambda: qk_unit(0, 1, 3)],
        (0, 3): [lambda tt=t: v_unit(tt) for t in range(12, 16)]
                + [lambda: qk_unit(1, 0, 0), lambda: qk_unit(1, 1, 0),
                   lambda: qk_unit(1, 0, 1), lambda: qk_unit(1, 1, 1)],
        (1, 0): [lambda: qk_unit(1, 0, 2), lambda: qk_unit(1, 1, 2)],
        (1, 1): [lambda: qk_unit(1, 0, 3), lambda: qk_unit(1, 1, 3),
                 lambda: qk_unit(2, 0, 0), lambda: qk_unit(2, 1, 0)],
        (1, 2): [lambda: qk_unit(2, 0, 1), lambda: qk_unit(2, 1, 1),
                 lambda: qk_unit(2, 0, 2), lambda: qk_unit(2, 1, 2)],
        (1, 3): [lambda: qk_unit(2, 0, 3), lambda: qk_unit(2, 1, 3)],
        (2, 0): [],
        (2, 1): [lambda tt=t: proj_unit(tt) for t in range(0, 4)],
        (2, 2): [lambda tt=t: proj_unit(tt) for t in range(4, 8)],
        (2, 3): [lambda tt=t: proj_unit(tt) for t in range(8, 12)],
    }

    # HAM warmup: dependency-free dummy matmuls stream while the input DMAs
    # land, so the PE clock gate is at 8/8 (2.4 GHz) when real work starts.
    wu = ps_t.tile([HS + 1, 64], F32, tag="tt", name="warmup")
    for _ in range(28):
        nc.tensor.matmul(wu, ones_rows, ones_rows[:, 0:64],
                         start=True, stop=True)

    # prefix: only pair-0 chunk-0 Q,K before attention starts
    qk_unit(0, 0, 0)
    qk_unit(0, 1, 0)

    pending_norm = []  # deferred normalization units

    for p in range(3):
        for m in range(NTC):
            jmax = 4 * m + 3
            last_chunk = (p == 2 and m == NTC - 1)
            fl = list(fillers[(p, m)])
            fl = pending_norm + fl
            pending_norm = []
            fi = 0

            PV_LAG = 3 if jmax >= 3 else (2 if jmax >= 2 else 1)

            def pv(jj):
                ss = max(0, jj - 4 * m)
                for e in range(2):
                    nc.tensor.matmul(
                        otu_ps[:, e, 128 * ss:512],
                        vaug[:, jj, 2 * p + e, :],
                        pts[jj][:, e, 128 * ss:512],
                        start=(jj == 0), stop=(jj == jmax),
                        skip_group_check=True,
                    )

            otu_ps = ps_otu.tile([HS + 1, 2, 512], F32, tag="otu", name="otu")
            pts = []
            for j in range(jmax + 1):
                s0 = max(0, j - 4 * m)
                st = ps_st.tile([128, 2, 512], F32, tag="st", name="st")
                for e in range(2):
                    nc.tensor.matmul(
                        st[:, e, 128 * s0:512],
                        pairK[p][64 * e:64 * e + 64, j * 128:(j + 1) * 128],
                        pairQ[p][64 * e:64 * e + 64,
                                 m * 512 + 128 * s0:(m + 1) * 512],
                        start=True, stop=True,
                        tile_position=(64 * e, 0),
                    )
                pt = ptp.tile([128, 2, 512], BF16, tag="pt", name="pt")
                pts.append(pt)
                nc.scalar.activation(
                    out=pt[:, :, 128 * s0:512],
                    in_=st[:, :, 128 * s0:512],
                    func=mybir.ActivationFunctionType.Exp,
                    scale=SCALE,
                )
                if j >= 4 * m:
                    # zero below-diagonal of the diagonal subtile (both
                    # heads) with a triangular-mask multiply on DVE.
                    nc.vector.tensor_mul(
                        out=pt[:, :, 128 * s0:128 * s0 + 128],
                        in0=pt[:, :, 128 * s0:128 * s0 + 128],
                        in1=trimask,
                    )
                # pace fillers evenly across the chunk
                while fi < len(fl) and (j + 1) * len(fl) >= (fi + 1) * (jmax + 1):
                    fl[fi]()
                    fi += 1
                # PV lags the exp by PV_LAG j-steps for pipeline slack
                if j >= PV_LAG:
                    pv(j - PV_LAG)
            # drain leftover fillers, then the last PVs
            while fi < len(fl):
                fl[fi]()
                fi += 1
            for jj in range(max(0, jmax + 1 - PV_LAG), jmax + 1):
                pv(jj)
            # reciprocal straight from PSUM at partition 0, then stash the
            # body to SBUF to free the psum for the next chunk; rb/mul are
            # deferred into the next chunk's filler slots.  For the last
            # chunk run the chain immediately and multiply straight out of
            # the psum.
            otu_sb = small.tile([HS + 1, 2, 512], F32, tag="otusb",
                                name="otusb")
            for e in range(2):
                nc.vector.reciprocal(out=otu_sb[HS:HS + 1, e, :],
                                     in_=otu_ps[HS:HS + 1, e, :])
            if last_chunk:
                state = (p, m, otu_sb, otu_ps, [])
                norm_rb(state)
                norm_mul(state, from_psum=True)
            else:
                nc.vector.tensor_copy(out=otu_sb[0:HS], in_=otu_ps[0:HS])
                state = (p, m, otu_sb, None, [])
                pending_norm = [lambda s=state: norm_rb(s),
                                lambda s=state: norm_mul(s)]

    # tail: final projection row
    for tt in range(12, 16):
        proj_unit(tt, tail=True)


_NC_CACHE = {}


def get_nc(repeat=1):
    key = repeat
    if key not in _NC_CACHE:
        nc = bacc.Bacc(
            "TRN2", target_bir_lowering=False, debug=False, num_devices=8
        )
        _NC_CACHE[key] = build_kernel(nc, repeat=repeat)
    return _NC_CACHE[key]


def make_in_maps(x, Wq, Wk, Wv, Wp, bp=None):
    x = np.asarray(x, dtype=np.float32)
    Wq = np.asarray(Wq, dtype=np.float32)
    Wk = np.asarray(Wk, dtype=np.float32)
    Wv = np.asarray(Wv, dtype=np.float32)
    Wp = np.asarray(Wp, dtype=np.float32)
    if bp is None:
        bp = np.zeros((C,), dtype=np.float32)
    bp = np.asarray(bp, dtype=np.float32)
    bf = ml_dtypes.bfloat16
    in_maps = []
    for c in range(8):
        b = c // 2
        hs = HL * (c % 2)
        xT = np.ascontiguousarray(x[b].T).astype(bf)
        # wqk[r, p, which, ci, col] = W_{q/k}[head pair p stacked][ci*128+r, col]
        wqk_ = np.empty((128, 3, 2, NCT, 128), dtype=bf)
        for p in range(3):
            sq = np.concatenate([Wq[hs + 2 * p], Wq[hs + 2 * p + 1]], axis=1)
            sk = np.concatenate([Wk[hs + 2 * p], Wk[hs + 2 * p + 1]], axis=1)
            wqk_[:, p, 0] = sq.reshape(NCT, 128, 128).transpose(1, 0, 2)
            wqk_[:, p, 1] = sk.reshape(NCT, 128, 128).transpose(1, 0, 2)
        wqk0_ = np.ascontiguousarray(wqk_[:, 0].reshape(128, -1))
        wqk12_ = np.ascontiguousarray(wqk_[:, 1:3].reshape(128, -1))
        wv_full = np.transpose(Wv[hs:hs + HL], (1, 0, 2)).reshape(C, HL * HS)
        wv_ = np.ascontiguousarray(
            wv_full.reshape(NCT, 128, HL * HS).transpose(1, 0, 2)
        ).astype(bf)
        wpt_ = np.ascontiguousarray(
            Wp[:, hs * HS:(hs + HL) * HS].T.reshape(3, 128, C).transpose(1, 0, 2)
        ).astype(bf)
        bpb_ = (bp.reshape(1, C) if c % 2 == 0
                else np.zeros((1, C), dtype=np.float32))
        in_maps.append({"xT": xT, "wqk0": wqk0_, "wqk12": wqk12_,
                        "wv": wv_.reshape(128, -1),
                        "wpt": wpt_.reshape(128, -1), "bpb": bpb_})
    return in_maps


def run(x, Wq, Wk, Wv, Wp, bp, trace=False):
    nc = get_nc()
    in_maps = make_in_maps(x, Wq, Wk, Wv, Wp, bp)
    res = bass_utils.run_bass_kernel_spmd(
        nc, in_maps, core_ids=list(range(8)), trace=trace
    )
    y = np.zeros((B, T, C), dtype=np.float32)
    for c in range(8):
        y[c // 2] += np.asarray(res.results[c]["y"], dtype=np.float32)
    return y, res


def kernel(x, Wq, Wk, Wv, Wp, bp):
    y, _ = run(x, Wq, Wk, Wv, Wp, bp)
    return y


def make_runner(nc):
    """Build the sharded PJRT callable once. Returns (fn, prep, zeros,
    out_names, make_loop_fn)."""
    import jax
    from jax.experimental.shard_map import shard_map
    from jax.sharding import Mesh, PartitionSpec, NamedSharding
    from concourse import mybir as _mybir
    from concourse.bass2jax import (
        _bass_exec_p, install_neuronx_cc_hook, partition_id_tensor,
    )

    install_neuronx_cc_hook()
    n_cores = 8
    partition_name = (
        nc.partition_id_tensor.name if nc.partition_id_tensor else None
    )
    in_names, out_names, out_avals = [], [], []
    for alloc in nc.m.functions[0].allocations:
        if not isinstance(alloc, _mybir.MemoryLocationSet):
            continue
        name = alloc.memorylocations[0].name
        if alloc.kind == "ExternalInput":
            if name != partition_name:
                in_names.append(name)
        elif alloc.kind == "ExternalOutput":
            out_names.append(name)
            out_avals.append(
                jax.core.ShapedArray(
                    tuple(alloc.tensor_shape), _mybir.dt.np(alloc.dtype)
                )
            )
    n_params = len(in_names)
    n_outs = len(out_avals)
    all_in_names = in_names + out_names
    if partition_name is not None:
        all_in_names.append(partition_name)

    def _body(*args):
        operands = list(args)
        if partition_name is not None:
            operands.append(partition_id_tensor())
        outs = _bass_exec_p.bind(
            *operands,
            out_avals=tuple(out_avals),
            in_names=tuple(all_in_names),
            out_names=tuple(out_names),
            lowering_input_output_aliases=(),
            sim_require_finite=True,
            sim_require_nnan=True,
            nc=nc,
        )
        return tuple(outs)

    devices = jax.devices()[:n_cores]
    mesh = Mesh(np.array(devices), ("core",))
    sharded = jax.jit(
        shard_map(
            _body, mesh=mesh,
            in_specs=(PartitionSpec("core"),) * (n_params + n_outs),
            out_specs=(PartitionSpec("core"),) * n_outs,
            check_rep=False,
        ),
        donate_argnums=tuple(range(n_params, n_params + n_outs)),
        keep_unused=True,
    )
    shd = NamedSharding(mesh, PartitionSpec("core"))

    def prep(in_maps):
        return [
            jax.device_put(
                np.concatenate([in_maps[c][nm] for c in range(n_cores)], axis=0),
                shd,
            )
            for nm in in_names
        ]

    def zeros():
        return [
            jax.device_put(
                np.zeros((n_cores * a.shape[0], *a.shape[1:]), a.dtype), shd
            )
            for a in out_avals
        ]

    def fn(dev_inputs, dev_zeros):
        outs = sharded(*dev_inputs, *dev_zeros)
        jax.block_until_ready(outs)
        return outs

    def make_loop_fn(n_iters):
        def _body_n(*args):
            ins = args[:n_params]
            carry = tuple(args[n_params:])

            def step(i, carry):
                operands = list(ins) + list(carry)
                if partition_name is not None:
                    operands.append(partition_id_tensor())
                outs = _bass_exec_p.bind(
                    *operands,
                    out_avals=tuple(out_avals),
                    in_names=tuple(all_in_names),
                    out_names=tuple(out_names),
                    lowering_input_output_aliases=(),
                    sim_require_finite=True,
                    sim_require_nnan=True,
                    nc=nc,
                )
                return tuple(outs)

            return jax.lax.fori_loop(0, n_iters, step, carry)

        looped = jax.jit(
            shard_map(
                _body_n, mesh=mesh,
                in_specs=(PartitionSpec("core"),) * (n_params + n_outs),
                out_specs=(PartitionSpec("core"),) * n_outs,
                check_rep=False,
            ),
            donate_argnums=tuple(range(n_params, n_params + n_outs)),
            keep_unused=True,
        )

        def run_n(dev_inputs, dev_zeros):
            outs = looped(*dev_inputs, *dev_zeros)
            jax.block_until_ready(outs)
            return outs

        return run_n

    return fn, prep, zeros, out_names, make_loop_fn


# revision 3
# speedup vs baseline: 1.0745x; 1.0745x over previous
"""Multi-head causal attention (B=4, T=2048, C=768, H=12, HS=64) on 8 trn2 cores.

v4 over the v2 baseline:
  - inputs host-packed partition-major so the whole weight set lands in 5
    large DMAs (was 37 small ones) and x in 12 half-band DMAs ordered so the
    first QK chunk waits only for the first halves; per-descriptor queue
    overhead drops ~5x.
  - diagonal causal masking via DVE multiply with a precomputed triangular
    mask (was gpsimd affine_select on the exp->PV critical chain).
  - output bias added on device (fused into the proj psum->sbuf copy on
    DVE; even cores get the real bias, odd cores zeros).
  - ysb pool deepened (3->6) so psum-recycle semaphores never chain tail
    projections behind recent y DMAs.
  - tail: the last chunk's normalization runs immediately and multiplies
    straight out of the otu psum.
  - PV lags the exp stream by 4 j-steps (deeper pt pool) for extra
    cross-engine pipeline slack on hardware.

Sharding: 48 (batch, head) units -> 6 per core. Core c: batch c//2, heads
6*(c%2) .. 6*(c%2)+6. Each core computes a partial output projection
y_partial[T, C]; host sums the two partials per batch.
"""

import numpy as np
import ml_dtypes

import concourse.bacc as bacc
import concourse.bass as bass
import concourse.tile as tile
from concourse import mybir
from concourse import bass_utils

B, T, C = 4, 2048, 768
H, HS = 12, 64
HL = 6            # heads per core
NCT = C // 128    # 6 contraction tiles
NTT = T // 128    # 16 t tiles
NTC = T // 512    # 4 t chunks
SCALE = 1.0 / 8.0  # 1/sqrt(HS)

F32 = mybir.dt.float32
BF16 = mybir.dt.bfloat16


def build_kernel(nc, repeat=1):
    xT = nc.dram_tensor("xT", [C, T], BF16, kind="ExternalInput").ap()
    # host-packed partition-major weights, flat per-partition rows so
    # every weight DMA lowers to a plain 2D descriptor
    wqk0 = nc.dram_tensor("wqk0", [128, 2 * NCT * 128], BF16,
                          kind="ExternalInput").ap()
    wqk12 = nc.dram_tensor("wqk12", [128, 2 * 2 * NCT * 128], BF16,
                           kind="ExternalInput").ap()
    wv = nc.dram_tensor("wv", [128, NCT * HL * HS], BF16,
                        kind="ExternalInput").ap()
    wpt = nc.dram_tensor("wpt", [128, 3 * C], BF16,
                         kind="ExternalInput").ap()
    bpb = nc.dram_tensor("bpb", [1, C], F32, kind="ExternalInput").ap()
    y = nc.dram_tensor("y", [T, C], BF16, kind="ExternalOutput").ap()

    with tile.TileContext(nc) as tc:
        with (
            tc.tile_pool(name="consts", bufs=1) as consts,
            tc.tile_pool(name="xw", bufs=1) as xw,
            tc.tile_pool(name="pt", bufs=20) as ptp,
            tc.tile_pool(name="small", bufs=4) as small,
            tc.tile_pool(name="ysb", bufs=6) as ysbp,
            # PSUM budget (8 banks): st 2x2 + otu 1x2 + tt 2x1
            tc.tile_pool(name="ps_st", bufs=2, space="PSUM") as ps_st,
            tc.tile_pool(name="ps_otu", bufs=1, space="PSUM") as ps_otu,
            tc.tile_pool(name="ps_t", bufs=2, space="PSUM") as ps_t,
        ):
            # ---------------- input DMAs ----------------
            # scalar queue: pair-0 QK weights (one DMA), then odd x halves.
            # sync queue: even x halves, then V/proj weights and the rest.
            # First-QK needs wqk[:, 0] + the h0 halves of every band.
            wqk0_sb = xw.tile([128, 2, NCT, 128], BF16, tag="wqk0",
                              name="wqk0")
            nc.scalar.dma_start(
                out=wqk0_sb.rearrange("p a b c -> p (a b c)"), in_=wqk0)
            xt = [xw.tile([128, T], BF16, tag=f"xt{ci}", name=f"xt{ci}")
                  for ci in range(NCT)]
            for h in range(2):
                for ci in range(NCT):
                    q = nc.sync if ci % 2 == 0 else nc.scalar
                    q.dma_start(
                        out=xt[ci][:, h * 1024:(h + 1) * 1024],
                        in_=xT[ci * 128:(ci + 1) * 128,
                               h * 1024:(h + 1) * 1024],
                    )
            wv_sb = xw.tile([128, NCT, HL * HS], BF16, tag="wv", name="wv")
            nc.sync.dma_start(
                out=wv_sb.rearrange("p a b -> p (a b)"), in_=wv)
            wqk12_sb = xw.tile([128, 2, 2, NCT, 128], BF16, tag="wqk12",
                               name="wqk12")
            nc.sync.dma_start(
                out=wqk12_sb.rearrange("p a b c d -> p (a b c d)"),
                in_=wqk12)
            wpt_sb = consts.tile([128, 3, C], BF16, tag="wpt", name="wpt")
            nc.sync.dma_start(
                out=wpt_sb.rearrange("p a b -> p (a b)"), in_=wpt)
            bprow = consts.tile([1, C], F32, tag="bprow", name="bprow")
            nc.scalar.dma_start(out=bprow, in_=bpb)

            # persistent tensors; augmented V column 0 carries the ones row
            # so the softmax denominator lands at psum partition 0.
            vaug = consts.tile([128, NTT, HL, HS + 1], BF16)
            nc.gpsimd.memset(vaug[:, :, :, HS:HS + 1], 1.0)
            pairQ = [consts.tile([128, T], BF16, tag=f"pq{p}", name=f"pq{p}")
                     for p in range(3)]
            pairK = [consts.tile([128, T], BF16, tag=f"pk{p}", name=f"pk{p}")
                     for p in range(3)]
            otn = consts.tile([128, 3, T], BF16)
            ones_rows = consts.tile([128, HS + 1], F32)
            nc.gpsimd.memset(ones_rows, 1.0)
            # upper-triangular (keep c >= r) mask for the diagonal subtiles
            trimask = consts.tile([128, 2, 128], BF16)
            nc.gpsimd.memset(trimask, 1.0)
            nc.gpsimd.affine_select(
                out=trimask, in_=trimask,
                compare_op=mybir.AluOpType.is_ge,
                fill=0.0, base=0,
                pattern=[[0, 2], [1, 128]],
                channel_multiplier=-1,
            )
            # bias broadcast to all 128 partitions (off the critical path)
            bias128 = consts.tile([128, C], F32)
            nc.gpsimd.partition_broadcast(bias128, bprow, channels=128)

            import contextlib
            rep_ctx = (
                tc.For_i(0, repeat, 1,
                         hint_engines=(mybir.EngineType.PE,
                                       mybir.EngineType.DVE,
                                       mybir.EngineType.Activation,
                                       mybir.EngineType.SP,
                                       mybir.EngineType.Pool))
                if repeat > 1 else contextlib.nullcontext()
            )
            with rep_ctx:
                build_phases(nc, tc, consts, xw, ptp, small, ysbp,
                             ps_st, ps_otu, ps_t,
                             xt, wqk0_sb, wqk12_sb, wv_sb, wpt_sb,
                             vaug, pairQ, pairK, otn, ones_rows,
                             trimask, bias128, y)

    nc.compile()
    return nc


def build_phases(nc, tc, consts, xw, ptp, small, ysbp,
                 ps_st, ps_otu, ps_t,
                 xt, wqk0_sb, wqk12_sb, wv_sb, wpt_sb,
                 vaug, pairQ, pairK, otn, ones_rows, trimask, bias128, y):
    def wqk_ap(p, which, ci):
        if p == 0:
            return wqk0_sb[:, which, ci, :]
        return wqk12_sb[:, p - 1, which, ci, :]

    # ---- filler unit builders (each issues one psum-group of PE work) ----
    def qk_unit(p, which, m):
        """Project one 512-col chunk of Q (which=0) or K (which=1) for pair p
        straight into pairQ/pairK (partition-aligned, no staging DMA)."""
        dst = pairQ[p] if which == 0 else pairK[p]
        sl = slice(m * 512, (m + 1) * 512)
        ps = ps_t.tile([128, 512], F32, tag="tt", name=f"qk{p}_{which}_{m}")
        for ci in range(NCT):
            nc.tensor.matmul(ps, wqk_ap(p, which, ci), xt[ci][:, sl],
                             start=(ci == 0), stop=(ci == NCT - 1))
        nc.vector.tensor_copy(out=dst[:, sl], in_=ps)

    def v_unit(tt):
        ps = ps_t.tile([128, HL * HS], F32, tag="tt", name=f"psv{tt}")
        for ci in range(NCT):
            nc.tensor.matmul(ps, xt[ci][:, tt * 128:(tt + 1) * 128],
                             wv_sb[:, ci, :],
                             start=(ci == 0), stop=(ci == NCT - 1))
        nc.vector.tensor_copy(
            out=vaug[:, tt, :, 0:HS],
            in_=ps.rearrange("p (h d) -> p h d", h=HL),
        )

    def norm_rb(state):
        """Row-broadcast of the reciprocal rows on GPSIMD.  The ucode reads
        literal partition 0, so hop the row down first with a tiny DMA."""
        p, m, otu_sb, otu_ps, rbs = state
        for e in range(2):
            stg = small.tile([1, 512], F32, tag="rstg", name=f"rs{p}_{m}_{e}")
            nc.sync.dma_start(out=stg, in_=otu_sb[HS:HS + 1, e, :])
            rb = small.tile([HS, 512], F32, tag="rbb", name=f"rb{p}_{m}_{e}")
            nc.gpsimd.partition_broadcast(rb, stg, channels=HS)
            rbs.append(rb)

    def norm_mul(state, from_psum=False):
        p, m, otu_sb, otu_ps, rbs = state
        for e in range(2):
            otnorm = small.tile([HS, 512], BF16, tag="otnorm", name="otnorm")
            body = (otu_ps if from_psum else otu_sb)[0:HS, e, :]
            nc.vector.tensor_mul(out=otnorm, in0=body, in1=rbs[e])
            nc.sync.dma_start(
                out=otn[64 * e:64 * e + HS, p, m * 512:(m + 1) * 512],
                in_=otnorm,
            )

    def proj_unit(tt, tail=False):
        y1 = ps_t.tile([128, 512], F32, tag="tt", name=f"y1_{tt}")
        y2 = ps_t.tile([128, 256], F32, tag="tt", name=f"y2_{tt}")
        for g in range(3):
            lhs = otn[:, g, tt * 128:(tt + 1) * 128]
            nc.tensor.matmul(y1, lhs, wpt_sb[:, g, 0:512],
                             start=(g == 0), stop=(g == 2))
            nc.tensor.matmul(y2, lhs, wpt_sb[:, g, 512:768],
                             start=(g == 0), stop=(g == 2))
        ysb = ysbp.tile([128, C], BF16, tag="ysb", name="ysb")
        # psum->sbuf copy fused with the device-side bias add; half-granular
        # so the store overlaps the second half.
        nc.vector.tensor_add(out=ysb[:, 0:512], in0=y1, in1=bias128[:, 0:512])
        nc.sync.dma_start(out=y[tt * 128:(tt + 1) * 128, 0:512],
                          in_=ysb[:, 0:512])
        nc.vector.tensor_add(out=ysb[:, 512:768], in0=y2,
                             in1=bias128[:, 512:768])
        q = nc.scalar if tail else nc.sync
        q.dma_start(out=y[tt * 128:(tt + 1) * 128, 512:768],
                    in_=ysb[:, 512:768])

    # ---- static filler schedule ----
    fillers = {
        (0, 0): [lambda tt=t: v_unit(tt) for t in range(0, 4)]
                + [lambda: qk_unit(0, 0, 1), lambda: qk_unit(0, 1, 1)],
        (0, 1): [lambda tt=t: v_unit(tt) for t in range(4, 8)]
                + [lambda: qk_unit(0, 0, 2), lambda: qk_unit(0, 1, 2)],
        (0, 2): [lambda tt=t: v_unit(tt) for t in range(8, 12)]
                + [lambda: qk_unit(0, 0, 3), lambda: qk_unit(0, 1, 3)],
        (0, 3): [lambda tt=t: v_unit(tt) for t in range(12, 16)]
                + [lambda: qk_unit(1, 0, 0), lambda: qk_unit(1, 1, 0),
                   lambda: qk_unit(1, 0, 1), lambda: qk_unit(1, 1, 1)],
        (1, 0): [lambda: qk_unit(1, 0, 2), lambda: qk_unit(1, 1, 2)],
        (1, 1): [lambda: qk_unit(1, 0, 3), lambda: qk_unit(1, 1, 3),
                 lambda: qk_unit(2, 0, 0), lambda: qk_unit(2, 1, 0)],
        (1, 2): [lambda: qk_unit(2, 0, 1), lambda: qk_unit(2, 1, 1),
                 lambda: qk_unit(2, 0, 2), lambda: qk_unit(2, 1, 2)],
        (1, 3): [lambda: qk_unit(2, 0, 3), lambda: qk_unit(2, 1, 3)],
        (2, 0): [],
        (2, 1): [lambda tt=t: proj_unit(tt) for t in range(0, 4)],
        (2, 2): [lambda tt=t: proj_unit(tt) for t in range(4, 8)],
        (2, 3): [lambda tt=t: proj_unit(tt) for t in range(8, 12)],
    }

    # HAM warmup: dependency-free dummy matmuls stream while the input DMAs
    # land, so the PE clock gate is at 8/8 (2.4 GHz) when real work starts.
    wu = ps_t.tile([HS + 1, 64], F32, tag="tt", name="warmup")
    for _ in range(28):
        nc.tensor.matmul(wu, ones_rows, ones_rows[:, 0:64],
                         start=True, stop=True)

    # prefix: only pair-0 chunk-0 Q,K before attention starts
    qk_unit(0, 0, 0)
    qk_unit(0, 1, 0)

    pending_norm = []  # deferred normalization units

    for p in range(3):
        for m in range(NTC):
            jmax = 4 * m + 3
            last_chunk = (p == 2 and m == NTC - 1)
            fl = list(fillers[(p, m)])
            fl = pending_norm + fl
            pending_norm = []
            fi = 0

            PV_LAG = 4 if jmax >= 4 else (jmax if jmax >= 1 else 1)

            def pv(jj):
                ss = max(0, jj - 4 * m)
                for e in range(2):
                    nc.tensor.matmul(
                        otu_ps[:, e, 128 * ss:512],
                        vaug[:, jj, 2 * p + e, :],
                        pts[jj][:, e, 128 * ss:512],
                        start=(jj == 0), stop=(jj == jmax),
                        skip_group_check=True,
                    )

            otu_ps = ps_otu.tile([HS + 1, 2, 512], F32, tag="otu", name="otu")
            pts = []
            for j in range(jmax + 1):
                s0 = max(0, j - 4 * m)
                st = ps_st.tile([128, 2, 512], F32, tag="st", name="st")
                for e in range(2):
                    nc.tensor.matmul(
                        st[:, e, 128 * s0:512],
                        pairK[p][64 * e:64 * e + 64, j * 128:(j + 1) * 128],
                        pairQ[p][64 * e:64 * e + 64,
                                 m * 512 + 128 * s0:(m + 1) * 512],
                        start=True, stop=True,
                        tile_position=(64 * e, 0),
                    )
                pt = ptp.tile([128, 2, 512], BF16, tag="pt", name="pt")
                pts.append(pt)
                nc.scalar.activation(
                    out=pt[:, :, 128 * s0:512],
                    in_=st[:, :, 128 * s0:512],
                    func=mybir.ActivationFunctionType.Exp,
                    scale=SCALE,
                )
                if j >= 4 * m:
                    # zero below-diagonal of the diagonal subtile (both
                    # heads) with a triangular-mask multiply on DVE.
                    nc.vector.tensor_mul(
                        out=pt[:, :, 128 * s0:128 * s0 + 128],
                        in0=pt[:, :, 128 * s0:128 * s0 + 128],
                        in1=trimask,
                    )
                # pace fillers evenly across the chunk
                while fi < len(fl) and (j + 1) * len(fl) >= (fi + 1) * (jmax + 1):
                    fl[fi]()
                    fi += 1
                # PV lags the exp by PV_LAG j-steps for pipeline slack
                if j >= PV_LAG:
                    pv(j - PV_LAG)
            # drain leftover fillers, then the last PVs
            while fi < len(fl):
                fl[fi]()
                fi += 1
            for jj in range(max(0, jmax + 1 - PV_LAG), jmax + 1):
                pv(jj)
            # reciprocal straight from PSUM at partition 0, then stash the
            # body to SBUF to free the psum for the next chunk; rb/mul are
            # deferred into the next chunk's filler slots.  For the last
            # chunk run the chain immediately and multiply straight out of
            # the psum.
            otu_sb = small.tile([HS + 1, 2, 512], F32, tag="otusb",
                                name="otusb")
            for e in range(2):
                nc.vector.reciprocal(out=otu_sb[HS:HS + 1, e, :],
                                     in_=otu_ps[HS:HS + 1, e, :])
            if last_chunk:
                state = (p, m, otu_sb, otu_ps, [])
                norm_rb(state)
                norm_mul(state, from_psum=True)
            else:
                nc.vector.tensor_copy(out=otu_sb[0:HS], in_=otu_ps[0:HS])
                state = (p, m, otu_sb, None, [])
                pending_norm = [lambda s=state: norm_rb(s),
                                lambda s=state: norm_mul(s)]

    # tail: final projection row
    for tt in range(12, 16):
        proj_unit(tt, tail=True)


_NC_CACHE = {}


def get_nc(repeat=1):
    key = repeat
    if key not in _NC_CACHE:
        nc = bacc.Bacc(
            "TRN2", target_bir_lowering=False, debug=False, num_devices=8
        )
        _NC_CACHE[key] = build_kernel(nc, repeat=repeat)
    return _NC_CACHE[key]


def make_in_maps(x, Wq, Wk, Wv, Wp, bp=None):
    x = np.asarray(x, dtype=np.float32)
    Wq = np.asarray(Wq, dtype=np.float32)
    Wk = np.asarray(Wk, dtype=np.float32)
    Wv = np.asarray(Wv, dtype=np.float32)
    Wp = np.asarray(Wp, dtype=np.float32)
    if bp is None:
        bp = np.zeros((C,), dtype=np.float32)
    bp = np.asarray(bp, dtype=np.float32)
    bf = ml_dtypes.bfloat16
    in_maps = []
    for c in range(8):
        b = c // 2
        hs = HL * (c % 2)
        xT = np.ascontiguousarray(x[b].T).astype(bf)
        # wqk[r, p, which, ci, col] = W_{q/k}[head pair p stacked][ci*128+r, col]
        wqk_ = np.empty((128, 3, 2, NCT, 128), dtype=bf)
        for p in range(3):
            sq = np.concatenate([Wq[hs + 2 * p], Wq[hs + 2 * p + 1]], axis=1)
            sk = np.concatenate([Wk[hs + 2 * p], Wk[hs + 2 * p + 1]], axis=1)
            wqk_[:, p, 0] = sq.reshape(NCT, 128, 128).transpose(1, 0, 2)
            wqk_[:, p, 1] = sk.reshape(NCT, 128, 128).transpose(1, 0, 2)
        wqk0_ = np.ascontiguousarray(wqk_[:, 0].reshape(128, -1))
        wqk12_ = np.ascontiguousarray(wqk_[:, 1:3].reshape(128, -1))
        wv_full = np.transpose(Wv[hs:hs + HL], (1, 0, 2)).reshape(C, HL * HS)
        wv_ = np.ascontiguousarray(
            wv_full.reshape(NCT, 128, HL * HS).transpose(1, 0, 2)
        ).astype(bf)
        wpt_ = np.ascontiguousarray(
            Wp[:, hs * HS:(hs + HL) * HS].T.reshape(3, 128, C).transpose(1, 0, 2)
        ).astype(bf)
        bpb_ = (bp.reshape(1, C) if c % 2 == 0
                else np.zeros((1, C), dtype=np.float32))
        in_maps.append({"xT": xT, "wqk0": wqk0_, "wqk12": wqk12_,
                        "wv": wv_.reshape(128, -1),
                        "wpt": wpt_.reshape(128, -1), "bpb": bpb_})
    return in_maps


def run(x, Wq, Wk, Wv, Wp, bp, trace=False):
    nc = get_nc()
    in_maps = make_in_maps(x, Wq, Wk, Wv, Wp, bp)
    res = bass_utils.run_bass_kernel_spmd(
        nc, in_maps, core_ids=list(range(8)), trace=trace
    )
    y = np.zeros((B, T, C), dtype=np.float32)
    for c in range(8):
        y[c // 2] += np.asarray(res.results[c]["y"], dtype=np.float32)
    return y, res


def kernel(x, Wq, Wk, Wv, Wp, bp):
    y, _ = run(x, Wq, Wk, Wv, Wp, bp)
    return y


def make_runner(nc):
    """Build the sharded PJRT callable once. Returns (fn, prep, zeros,
    out_names, make_loop_fn)."""
    import jax
    from jax.experimental.shard_map import shard_map
    from jax.sharding import Mesh, PartitionSpec, NamedSharding
    from concourse import mybir as _mybir
    from concourse.bass2jax import (
        _bass_exec_p, install_neuronx_cc_hook, partition_id_tensor,
    )

    install_neuronx_cc_hook()
    n_cores = 8
    partition_name = (
        nc.partition_id_tensor.name if nc.partition_id_tensor else None
    )
    in_names, out_names, out_avals = [], [], []
    for alloc in nc.m.functions[0].allocations:
        if not isinstance(alloc, _mybir.MemoryLocationSet):
            continue
        name = alloc.memorylocations[0].name
        if alloc.kind == "ExternalInput":
            if name != partition_name:
                in_names.append(name)
        elif alloc.kind == "ExternalOutput":
            out_names.append(name)
            out_avals.append(
                jax.core.ShapedArray(
                    tuple(alloc.tensor_shape), _mybir.dt.np(alloc.dtype)
                )
            )
    n_params = len(in_names)
    n_outs = len(out_avals)
    all_in_names = in_names + out_names
    if partition_name is not None:
        all_in_names.append(partition_name)

    def _body(*args):
        operands = list(args)
        if partition_name is not None:
            operands.append(partition_id_tensor())
        outs = _bass_exec_p.bind(
            *operands,
            out_avals=tuple(out_avals),
            in_names=tuple(all_in_names),
            out_names=tuple(out_names),
            lowering_input_output_aliases=(),
            sim_require_finite=True,
            sim_require_nnan=True,
            nc=nc,
        )
        return tuple(outs)

    devices = jax.devices()[:n_cores]
    mesh = Mesh(np.array(devices), ("core",))
    sharded = jax.jit(
        shard_map(
            _body, mesh=mesh,
            in_specs=(PartitionSpec("core"),) * (n_params + n_outs),
            out_specs=(PartitionSpec("core"),) * n_outs,
            check_rep=False,
        ),
        donate_argnums=tuple(range(n_params, n_params + n_outs)),
        keep_unused=True,
    )
    shd = NamedSharding(mesh, PartitionSpec("core"))

    def prep(in_maps):
        return [
            jax.device_put(
                np.concatenate([in_maps[c][nm] for c in range(n_cores)], axis=0),
                shd,
            )
            for nm in in_names
        ]

    def zeros():
        return [
            jax.device_put(
                np.zeros((n_cores * a.shape[0], *a.shape[1:]), a.dtype), shd
            )
            for a in out_avals
        ]

    def fn(dev_inputs, dev_zeros):
        outs = sharded(*dev_inputs, *dev_zeros)
        jax.block_until_ready(outs)
        return outs

    def make_loop_fn(n_iters):
        def _body_n(*args):
            ins = args[:n_params]
            carry = tuple(args[n_params:])

            def step(i, carry):
                operands = list(ins) + list(carry)
                if partition_name is not None:
                    operands.append(partition_id_tensor())
                outs = _bass_exec_p.bind(
                    *operands,
                    out_avals=tuple(out_avals),
                    in_names=tuple(all_in_names),
                    out_names=tuple(out_names),
                    lowering_input_output_aliases=(),
                    sim_require_finite=True,
                    sim_require_nnan=True,
                    nc=nc,
                )
                return tuple(outs)

            return jax.lax.fori_loop(0, n_iters, step, carry)

        looped = jax.jit(
            shard_map(
                _body_n, mesh=mesh,
                in_specs=(PartitionSpec("core"),) * (n_params + n_outs),
                out_specs=(PartitionSpec("core"),) * n_outs,
                check_rep=False,
            ),
            donate_argnums=tuple(range(n_params, n_params + n_outs)),
            keep_unused=True,
        )

        def run_n(dev_inputs, dev_zeros):
            outs = looped(*dev_inputs, *dev_zeros)
            jax.block_until_ready(outs)
            return outs

        return run_n

    return fn, prep, zeros, out_names, make_loop_fn


# revision 6
# speedup vs baseline: 1.2869x; 1.1977x over previous
"""Multi-head causal attention (B=4, T=2048, C=768, H=12, HS=64) on 8 trn2 cores.

v4 over the v2 baseline:
  - inputs host-packed partition-major so the whole weight set lands in 5
    large DMAs (was 37 small ones) and x in 12 half-band DMAs ordered so the
    first QK chunk waits only for the first halves; per-descriptor queue
    overhead drops ~5x.
  - diagonal causal masking via DVE multiply with a precomputed triangular
    mask (was gpsimd affine_select on the exp->PV critical chain).
  - output bias added on device (fused into the proj psum->sbuf copy on
    DVE; even cores get the real bias, odd cores zeros).
  - ysb pool deepened (3->6) so psum-recycle semaphores never chain tail
    projections behind recent y DMAs.
  - tail: the last chunk's normalization runs immediately and multiplies
    straight out of the otu psum.
  - PV lags the exp stream by 5 j-steps (deeper pt pool) and the small
    pool is deepened so Tile's pool-recycle semaphores never chain the PE
    behind recent DMAs; pair-2's last QK fillers fill the thin (2,0) chunk.

Sharding: 48 (batch, head) units -> 6 per core. Core c: batch c//2, heads
6*(c%2) .. 6*(c%2)+6. Each core computes a partial output projection
y_partial[T, C]; host sums the two partials per batch.
"""

import numpy as np
import ml_dtypes

import concourse.bacc as bacc
import concourse.bass as bass
import concourse.tile as tile
from concourse import mybir
from concourse import bass_utils

B, T, C = 4, 2048, 768
H, HS = 12, 64
HL = 6            # heads per core
NCT = C // 128    # 6 contraction tiles
NTT = T // 128    # 16 t tiles
NTC = T // 512    # 4 t chunks
SCALE = 1.0 / 8.0  # 1/sqrt(HS)

F32 = mybir.dt.float32
BF16 = mybir.dt.bfloat16


def build_kernel(nc, repeat=1):
    xT = nc.dram_tensor("xT", [C, T], BF16, kind="ExternalInput").ap()
    # host-packed partition-major weights, flat per-partition rows so
    # every weight DMA lowers to a plain 2D descriptor
    wqk0 = nc.dram_tensor("wqk0", [128, 2 * NCT * 128], BF16,
                          kind="ExternalInput").ap()
    wqk12 = nc.dram_tensor("wqk12", [128, 2 * 2 * NCT * 128], BF16,
                           kind="ExternalInput").ap()
    wv = nc.dram_tensor("wv", [128, NCT * HL * HS], BF16,
                        kind="ExternalInput").ap()
    wpt = nc.dram_tensor("wpt", [128, 3 * C], BF16,
                         kind="ExternalInput").ap()
    bpb = nc.dram_tensor("bpb", [1, C], F32, kind="ExternalInput").ap()
    y = nc.dram_tensor("y", [T, C], BF16, kind="ExternalOutput").ap()

    with tile.TileContext(nc) as tc:
        with (
            tc.tile_pool(name="consts", bufs=1) as consts,
            tc.tile_pool(name="xw", bufs=1) as xw,
            tc.tile_pool(name="pt", bufs=21) as ptp,
            tc.tile_pool(name="small", bufs=6) as small,
            tc.tile_pool(name="ysb", bufs=6) as ysbp,
            # PSUM budget (8 banks): st 2x2 + otu 1x2 + tt 2x1
            tc.tile_pool(name="ps_st", bufs=2, space="PSUM") as ps_st,
            tc.tile_pool(name="ps_otu", bufs=1, space="PSUM") as ps_otu,
            tc.tile_pool(name="ps_t", bufs=2, space="PSUM") as ps_t,
        ):
            # ---------------- input DMAs ----------------
            # scalar queue: pair-0 QK weights (one DMA), then odd x halves.
            # sync queue: even x halves, then V/proj weights and the rest.
            # First-QK needs wqk[:, 0] + the h0 halves of every band.
            wqk0_sb = xw.tile([128, 2, NCT, 128], BF16, tag="wqk0",
                              name="wqk0")
            nc.scalar.dma_start(
                out=wqk0_sb.rearrange("p a b c -> p (a b c)"), in_=wqk0)
            xt = [xw.tile([128, T], BF16, tag=f"xt{ci}", name=f"xt{ci}")
                  for ci in range(NCT)]
            for h in range(2):
                for ci in range(NCT):
                    q = nc.sync if ci % 2 == 0 else nc.scalar
                    q.dma_start(
                        out=xt[ci][:, h * 1024:(h + 1) * 1024],
                        in_=xT[ci * 128:(ci + 1) * 128,
                               h * 1024:(h + 1) * 1024],
                    )
            wv_sb = xw.tile([128, NCT, HL * HS], BF16, tag="wv", name="wv")
            nc.sync.dma_start(
                out=wv_sb.rearrange("p a b -> p (a b)"), in_=wv)
            wqk12_sb = xw.tile([128, 2, 2, NCT, 128], BF16, tag="wqk12",
                               name="wqk12")
            nc.sync.dma_start(
                out=wqk12_sb.rearrange("p a b c d -> p (a b c d)"),
                in_=wqk12)
            wpt_sb = consts.tile([128, 3, C], BF16, tag="wpt", name="wpt")
            nc.sync.dma_start(
                out=wpt_sb.rearrange("p a b -> p (a b)"), in_=wpt)
            bprow = consts.tile([1, C], F32, tag="bprow", name="bprow")
            nc.scalar.dma_start(out=bprow, in_=bpb)

            # persistent tensors; augmented V column 0 carries the ones row
            # so the softmax denominator lands at psum partition 0.
            vaug = consts.tile([128, NTT, HL, HS + 1], BF16)
            nc.gpsimd.memset(vaug[:, :, :, HS:HS + 1], 1.0)
            pairQ = [consts.tile([128, T], BF16, tag=f"pq{p}", name=f"pq{p}")
                     for p in range(3)]
            pairK = [consts.tile([128, T], BF16, tag=f"pk{p}", name=f"pk{p}")
                     for p in range(3)]
            otn = consts.tile([128, 3, T], BF16)
            ones_rows = consts.tile([128, HS + 1], BF16)
            nc.gpsimd.memset(ones_rows, 1.0)
            # upper-triangular (keep c >= r) mask for the diagonal subtiles
            trimask = consts.tile([128, 2, 128], BF16)
            nc.gpsimd.memset(trimask, 1.0)
            nc.gpsimd.affine_select(
                out=trimask, in_=trimask,
                compare_op=mybir.AluOpType.is_ge,
                fill=0.0, base=0,
                pattern=[[0, 2], [1, 128]],
                channel_multiplier=-1,
            )
            # bias broadcast to all 128 partitions (off the critical path)
            bias128 = consts.tile([128, C], F32)
            nc.gpsimd.partition_broadcast(bias128, bprow, channels=128)

            import contextlib
            rep_ctx = (
                tc.For_i(0, repeat, 1,
                         hint_engines=(mybir.EngineType.PE,
                                       mybir.EngineType.DVE,
                                       mybir.EngineType.Activation,
                                       mybir.EngineType.SP,
                                       mybir.EngineType.Pool))
                if repeat > 1 else contextlib.nullcontext()
            )
            with rep_ctx:
                build_phases(nc, tc, consts, xw, ptp, small, ysbp,
                             ps_st, ps_otu, ps_t,
                             xt, wqk0_sb, wqk12_sb, wv_sb, wpt_sb,
                             vaug, pairQ, pairK, otn, ones_rows,
                             trimask, bias128, y)

    nc.compile()
    return nc


def build_phases(nc, tc, consts, xw, ptp, small, ysbp,
                 ps_st, ps_otu, ps_t,
                 xt, wqk0_sb, wqk12_sb, wv_sb, wpt_sb,
                 vaug, pairQ, pairK, otn, ones_rows, trimask, bias128, y):
    def wqk_ap(p, which, ci):
        if p == 0:
            return wqk0_sb[:, which, ci, :]
        return wqk12_sb[:, p - 1, which, ci, :]

    # ---- filler unit builders (each issues one psum-group of PE work) ----
    def qk_unit(p, which, m):
        """Project one 512-col chunk of Q (which=0) or K (which=1) for pair p
        straight into pairQ/pairK (partition-aligned, no staging DMA)."""
        dst = pairQ[p] if which == 0 else pairK[p]
        sl = slice(m * 512, (m + 1) * 512)
        ps = ps_t.tile([128, 512], F32, tag="tt", name=f"qk{p}_{which}_{m}")
        for ci in range(NCT):
            nc.tensor.matmul(ps, wqk_ap(p, which, ci), xt[ci][:, sl],
                             start=(ci == 0), stop=(ci == NCT - 1))
        nc.vector.tensor_copy(out=dst[:, sl], in_=ps)

    def v_unit(tt):
        ps = ps_t.tile([128, HL * HS], F32, tag="tt", name=f"psv{tt}")
        for ci in range(NCT):
            nc.tensor.matmul(ps, xt[ci][:, tt * 128:(tt + 1) * 128],
                             wv_sb[:, ci, :],
                             start=(ci == 0), stop=(ci == NCT - 1))
        nc.vector.tensor_copy(
            out=vaug[:, tt, :, 0:HS],
            in_=ps.rearrange("p (h d) -> p h d", h=HL),
        )

    def norm_rb(state):
        """Row-broadcast of the reciprocal rows on GPSIMD.  The ucode reads
        literal partition 0, so hop the row down first with a tiny DMA."""
        p, m, otu_sb, otu_ps, rbs = state
        for e in range(2):
            stg = small.tile([1, 512], F32, tag="rstg", name=f"rs{p}_{m}_{e}")
            nc.sync.dma_start(out=stg, in_=otu_sb[HS:HS + 1, e, :])
            rb = small.tile([HS, 512], F32, tag="rbb", name=f"rb{p}_{m}_{e}")
            nc.gpsimd.partition_broadcast(rb, stg, channels=HS)
            rbs.append(rb)

    def norm_mul(state, from_psum=False):
        p, m, otu_sb, otu_ps, rbs = state
        for e in range(2):
            otnorm = small.tile([HS, 512], BF16, tag="otnorm", name="otnorm")
            body = (otu_ps if from_psum else otu_sb)[0:HS, e, :]
            nc.vector.tensor_mul(out=otnorm, in0=body, in1=rbs[e])
            nc.sync.dma_start(
                out=otn[64 * e:64 * e + HS, p, m * 512:(m + 1) * 512],
                in_=otnorm,
            )

    def proj_unit(tt, tail=False):
        y1 = ps_t.tile([128, 512], F32, tag="tt", name=f"y1_{tt}")
        y2 = ps_t.tile([128, 256], F32, tag="tt", name=f"y2_{tt}")
        for g in range(3):
            lhs = otn[:, g, tt * 128:(tt + 1) * 128]
            nc.tensor.matmul(y1, lhs, wpt_sb[:, g, 0:512],
                             start=(g == 0), stop=(g == 2))
            nc.tensor.matmul(y2, lhs, wpt_sb[:, g, 512:768],
                             start=(g == 0), stop=(g == 2))
        ysb = ysbp.tile([128, C], BF16, tag="ysb", name="ysb")
        # psum->sbuf copy fused with the device-side bias add; half-granular
        # so the store overlaps the second half.
        nc.vector.tensor_add(out=ysb[:, 0:512], in0=y1, in1=bias128[:, 0:512])
        nc.sync.dma_start(out=y[tt * 128:(tt + 1) * 128, 0:512],
                          in_=ysb[:, 0:512])
        nc.vector.tensor_add(out=ysb[:, 512:768], in0=y2,
                             in1=bias128[:, 512:768])
        q = nc.scalar if tail else nc.sync
        q.dma_start(out=y[tt * 128:(tt + 1) * 128, 512:768],
                    in_=ysb[:, 512:768])

    # ---- static filler schedule ----
    fillers = {
        (0, 0): [lambda tt=t: v_unit(tt) for t in range(0, 4)]
                + [lambda: qk_unit(0, 0, 1), lambda: qk_unit(0, 1, 1)],
        (0, 1): [lambda tt=t: v_unit(tt) for t in range(4, 8)]
                + [lambda: qk_unit(0, 0, 2), lambda: qk_unit(0, 1, 2)],
        (0, 2): [lambda tt=t: v_unit(tt) for t in range(8, 12)]
                + [lambda: qk_unit(0, 0, 3), lambda: qk_unit(0, 1, 3)],
        (0, 3): [lambda tt=t: v_unit(tt) for t in range(12, 16)]
                + [lambda: qk_unit(1, 0, 0), lambda: qk_unit(1, 1, 0),
                   lambda: qk_unit(1, 0, 1), lambda: qk_unit(1, 1, 1)],
        (1, 0): [lambda: qk_unit(1, 0, 2), lambda: qk_unit(1, 1, 2)],
        (1, 1): [lambda: qk_unit(1, 0, 3), lambda: qk_unit(1, 1, 3),
                 lambda: qk_unit(2, 0, 0), lambda: qk_unit(2, 1, 0)],
        (1, 2): [lambda: qk_unit(2, 0, 1), lambda: qk_unit(2, 1, 1),
                 lambda: qk_unit(2, 0, 2), lambda: qk_unit(2, 1, 2)],
        (1, 3): [],
        (2, 0): [lambda: qk_unit(2, 0, 3), lambda: qk_unit(2, 1, 3)],
        (2, 1): [lambda tt=t: proj_unit(tt) for t in range(0, 4)],
        (2, 2): [lambda tt=t: proj_unit(tt) for t in range(4, 8)],
        (2, 3): [lambda tt=t: proj_unit(tt) for t in range(8, 12)],
    }

    # HAM warmup: dependency-free dummy matmuls stream while the input DMAs
    # land, so the PE clock gate is at 8/8 (2.4 GHz) when real work starts.
    wu = ps_t.tile([HS + 1, 64], F32, tag="tt", name="warmup")
    for _ in range(28):
        nc.tensor.matmul(wu, ones_rows, ones_rows[:, 0:64],
                         start=True, stop=True)

    # prefix: only pair-0 chunk-0 Q,K before attention starts
    qk_unit(0, 0, 0)
    qk_unit(0, 1, 0)

    pending_norm = []  # deferred normalization units

    for p in range(3):
        for m in range(NTC):
            jmax = 4 * m + 3
            last_chunk = (p == 2 and m == NTC - 1)
            fl = list(fillers[(p, m)])
            fl = pending_norm + fl
            pending_norm = []
            fi = 0

            PV_LAG = 5 if jmax >= 5 else (jmax if jmax >= 1 else 1)

            def pv(jj):
                ss = max(0, jj - 4 * m)
                for e in range(2):
                    nc.tensor.matmul(
                        otu_ps[:, e, 128 * ss:512],
                        vaug[:, jj, 2 * p + e, :],
                        pts[jj][:, e, 128 * ss:512],
                        start=(jj == 0), stop=(jj == jmax),
                        skip_group_check=True,
                    )

            otu_ps = ps_otu.tile([HS + 1, 2, 512], F32, tag="otu", name="otu")
            pts = []
            for j in range(jmax + 1):
                s0 = max(0, j - 4 * m)
                st = ps_st.tile([128, 2, 512], F32, tag="st", name="st")
                for e in range(2):
                    nc.tensor.matmul(
                        st[:, e, 128 * s0:512],
                        pairK[p][64 * e:64 * e + 64, j * 128:(j + 1) * 128],
                        pairQ[p][64 * e:64 * e + 64,
                                 m * 512 + 128 * s0:(m + 1) * 512],
                        start=True, stop=True,
                        tile_position=(64 * e, 0),
                    )
                pt = ptp.tile([128, 2, 512], BF16, tag="pt", name="pt")
                pts.append(pt)
                nc.scalar.activation(
                    out=pt[:, :, 128 * s0:512],
                    in_=st[:, :, 128 * s0:512],
                    func=mybir.ActivationFunctionType.Exp,
                    scale=SCALE,
                )
                if j >= 4 * m:
                    # zero below-diagonal of the diagonal subtile (both
                    # heads) with a triangular-mask multiply on DVE.
                    nc.vector.tensor_mul(
                        out=pt[:, :, 128 * s0:128 * s0 + 128],
                        in0=pt[:, :, 128 * s0:128 * s0 + 128],
                        in1=trimask,
                    )
                # pace fillers evenly across the chunk
                while fi < len(fl) and (j + 1) * len(fl) >= (fi + 1) * (jmax + 1):
                    fl[fi]()
                    fi += 1
                # PV lags the exp by PV_LAG j-steps for pipeline slack
                if j >= PV_LAG:
                    pv(j - PV_LAG)
            # drain leftover fillers, then the last PVs
            while fi < len(fl):
                fl[fi]()
                fi += 1
            for jj in range(max(0, jmax + 1 - PV_LAG), jmax + 1):
                pv(jj)
            # reciprocal straight from PSUM at partition 0, then stash the
            # body to SBUF to free the psum for the next chunk; rb/mul are
            # deferred into the next chunk's filler slots.  For the last
            # chunk run the chain immediately and multiply straight out of
            # the psum.
            otu_sb = small.tile([HS + 1, 2, 512], F32, tag="otusb",
                                name="otusb")
            for e in range(2):
                nc.vector.reciprocal(out=otu_sb[HS:HS + 1, e, :],
                                     in_=otu_ps[HS:HS + 1, e, :])
            if last_chunk:
                state = (p, m, otu_sb, otu_ps, [])
                norm_rb(state)
                norm_mul(state, from_psum=True)
            else:
                nc.vector.tensor_copy(out=otu_sb[0:HS], in_=otu_ps[0:HS])
                state = (p, m, otu_sb, None, [])
                pending_norm = [lambda s=state: norm_rb(s),
                                lambda s=state: norm_mul(s)]

    # tail: final projection row
    for tt in range(12, 16):
        proj_unit(tt, tail=True)


_NC_CACHE = {}


def get_nc(repeat=1):
    key = repeat
    if key not in _NC_CACHE:
        nc = bacc.Bacc(
            "TRN2", target_bir_lowering=False, debug=False, num_devices=8
        )
        _NC_CACHE[key] = build_kernel(nc, repeat=repeat)
    return _NC_CACHE[key]


def make_in_maps(x, Wq, Wk, Wv, Wp, bp=None):
    x = np.asarray(x, dtype=np.float32)
    Wq = np.asarray(Wq, dtype=np.float32)
    Wk = np.asarray(Wk, dtype=np.float32)
    Wv = np.asarray(Wv, dtype=np.float32)
    Wp = np.asarray(Wp, dtype=np.float32)
    if bp is None:
        bp = np.zeros((C,), dtype=np.float32)
    bp = np.asarray(bp, dtype=np.float32)
    bf = ml_dtypes.bfloat16
    # the two cores of a batch pair share xT; the four cores of a head
    # parity share all weights -- build each distinct block once
    xTs = {b: np.ascontiguousarray(x[b].T).astype(bf) for b in range(B)}
    wsets = {}
    for par in range(2):
        hs = HL * par
        # wqk[r, p, which, ci, col] = W_{q/k}[head pair p stacked][ci*128+r, col]
        wqk_ = np.empty((128, 3, 2, NCT, 128), dtype=bf)
        for p in range(3):
            sq = np.concatenate([Wq[hs + 2 * p], Wq[hs + 2 * p + 1]], axis=1)
            sk = np.concatenate([Wk[hs + 2 * p], Wk[hs + 2 * p + 1]], axis=1)
            wqk_[:, p, 0] = sq.reshape(NCT, 128, 128).transpose(1, 0, 2)
            wqk_[:, p, 1] = sk.reshape(NCT, 128, 128).transpose(1, 0, 2)
        wv_full = np.transpose(Wv[hs:hs + HL], (1, 0, 2)).reshape(C, HL * HS)
        wv_ = np.ascontiguousarray(
            wv_full.reshape(NCT, 128, HL * HS).transpose(1, 0, 2)
        ).astype(bf)
        wpt_ = np.ascontiguousarray(
            Wp[:, hs * HS:(hs + HL) * HS].T.reshape(3, 128, C).transpose(1, 0, 2)
        ).astype(bf)
        wsets[par] = {
            "wqk0": np.ascontiguousarray(wqk_[:, 0].reshape(128, -1)),
            "wqk12": np.ascontiguousarray(wqk_[:, 1:3].reshape(128, -1)),
            "wv": wv_.reshape(128, -1),
            "wpt": wpt_.reshape(128, -1),
        }
    bpb0 = bp.reshape(1, C).astype(np.float32)
    bpb1 = np.zeros((1, C), dtype=np.float32)
    in_maps = []
    for c in range(8):
        in_maps.append({"xT": xTs[c // 2],
                        **wsets[c % 2],
                        "bpb": bpb0 if c % 2 == 0 else bpb1})
    return in_maps


def run(x, Wq, Wk, Wv, Wp, bp, trace=False):
    nc = get_nc()
    in_maps = make_in_maps(x, Wq, Wk, Wv, Wp, bp)
    res = bass_utils.run_bass_kernel_spmd(
        nc, in_maps, core_ids=list(range(8)), trace=trace
    )
    y = np.zeros((B, T, C), dtype=np.float32)
    for c in range(8):
        y[c // 2] += np.asarray(res.results[c]["y"], dtype=np.float32)
    return y, res


def kernel(x, Wq, Wk, Wv, Wp, bp):
    y, _ = run(x, Wq, Wk, Wv, Wp, bp)
    return y


def make_runner(nc):
    """Build the sharded PJRT callable once. Returns (fn, prep, zeros,
    out_names, make_loop_fn)."""
    import jax
    from jax.experimental.shard_map import shard_map
    from jax.sharding import Mesh, PartitionSpec, NamedSharding
    from concourse import mybir as _mybir
    from concourse.bass2jax import (
        _bass_exec_p, install_neuronx_cc_hook, partition_id_tensor,
    )

    install_neuronx_cc_hook()
    n_cores = 8
    partition_name = (
        nc.partition_id_tensor.name if nc.partition_id_tensor else None
    )
    in_names, out_names, out_avals = [], [], []
    for alloc in nc.m.functions[0].allocations:
        if not isinstance(alloc, _mybir.MemoryLocationSet):
            continue
        name = alloc.memorylocations[0].name
        if alloc.kind == "ExternalInput":
            if name != partition_name:
                in_names.append(name)
        elif alloc.kind == "ExternalOutput":
            out_names.append(name)
            out_avals.append(
                jax.core.ShapedArray(
                    tuple(alloc.tensor_shape), _mybir.dt.np(alloc.dtype)
                )
            )
    n_params = len(in_names)
    n_outs = len(out_avals)
    all_in_names = in_names + out_names
    if partition_name is not None:
        all_in_names.append(partition_name)

    def _body(*args):
        operands = list(args)
        if partition_name is not None:
            operands.append(partition_id_tensor())
        outs = _bass_exec_p.bind(
            *operands,
            out_avals=tuple(out_avals),
            in_names=tuple(all_in_names),
            out_names=tuple(out_names),
            lowering_input_output_aliases=(),
            sim_require_finite=True,
            sim_require_nnan=True,
            nc=nc,
        )
        return tuple(outs)

    devices = jax.devices()[:n_cores]
    mesh = Mesh(np.array(devices), ("core",))
    sharded = jax.jit(
        shard_map(
            _body, mesh=mesh,
            in_specs=(PartitionSpec("core"),) * (n_params + n_outs),
            out_specs=(PartitionSpec("core"),) * n_outs,
            check_rep=False,
        ),
        donate_argnums=tuple(range(n_params, n_params + n_outs)),
        keep_unused=True,
    )
    shd = NamedSharding(mesh, PartitionSpec("core"))

    def prep(in_maps):
        return [
            jax.device_put(
                np.concatenate([in_maps[c][nm] for c in range(n_cores)], axis=0),
                shd,
            )
            for nm in in_names
        ]

    def zeros():
        return [
            jax.device_put(
                np.zeros((n_cores * a.shape[0], *a.shape[1:]), a.dtype), shd
            )
            for a in out_avals
        ]

    def fn(dev_inputs, dev_zeros):
        outs = sharded(*dev_inputs, *dev_zeros)
        jax.block_until_ready(outs)
        return outs

    def make_loop_fn(n_iters):
        def _body_n(*args):
            ins = args[:n_params]
            carry = tuple(args[n_params:])

            def step(i, carry):
                operands = list(ins) + list(carry)
                if partition_name is not None:
                    operands.append(partition_id_tensor())
                outs = _bass_exec_p.bind(
                    *operands,
                    out_avals=tuple(out_avals),
                    in_names=tuple(all_in_names),
                    out_names=tuple(out_names),
                    lowering_input_output_aliases=(),
                    sim_require_finite=True,
                    sim_require_nnan=True,
                    nc=nc,
                )
                return tuple(outs)

            return jax.lax.fori_loop(0, n_iters, step, carry)

        looped = jax.jit(
            shard_map(
                _body_n, mesh=mesh,
                in_specs=(PartitionSpec("core"),) * (n_params + n_outs),
                out_specs=(PartitionSpec("core"),) * n_outs,
                check_rep=False,
            ),
            donate_argnums=tuple(range(n_params, n_params + n_outs)),
            keep_unused=True,
        )

        def run_n(dev_inputs, dev_zeros):
            outs = looped(*dev_inputs, *dev_zeros)
            jax.block_until_ready(outs)
            return outs

        return run_n

    return fn, prep, zeros, out_names, make_loop_fn


# revision 7
# speedup vs baseline: 2.8448x; 2.2106x over previous
"""Multi-head causal attention (B=4, T=2048, C=768, H=12, HS=64) on 8 trn2 cores.

v4 over the v2 baseline:
  - inputs host-packed partition-major so the whole weight set lands in 5
    large DMAs (was 37 small ones) and x in 12 half-band DMAs ordered so the
    first QK chunk waits only for the first halves; per-descriptor queue
    overhead drops ~5x.
  - diagonal causal masking via DVE multiply with a precomputed triangular
    mask (was gpsimd affine_select on the exp->PV critical chain).
  - output bias added on device (fused into the proj psum->sbuf copy on
    DVE; even cores get the real bias, odd cores zeros).
  - ysb pool deepened (3->6) so psum-recycle semaphores never chain tail
    projections behind recent y DMAs.
  - tail: the last chunk's normalization runs immediately and multiplies
    straight out of the otu psum.
  - PV lags the exp stream by 5 j-steps (deeper pt pool) and the small
    pool is deepened so Tile's pool-recycle semaphores never chain the PE
    behind recent DMAs; pair-2's last QK fillers fill the thin (2,0) chunk.

Sharding: 48 (batch, head) units -> 6 per core. Core c: batch c//2, heads
6*(c%2) .. 6*(c%2)+6. Each core computes a partial output projection
y_partial[T, C]; host sums the two partials per batch.
"""

import numpy as np
import ml_dtypes

import concourse.bacc as bacc
import concourse.bass as bass
import concourse.tile as tile
from concourse import mybir
from concourse import bass_utils

B, T, C = 4, 2048, 768
H, HS = 12, 64
HL = 6            # heads per core
NCT = C // 128    # 6 contraction tiles
NTT = T // 128    # 16 t tiles
NTC = T // 512    # 4 t chunks
SCALE = 1.0 / 8.0  # 1/sqrt(HS)

F32 = mybir.dt.float32
BF16 = mybir.dt.bfloat16


def build_kernel(nc, repeat=1):
    xT = nc.dram_tensor("xT", [C, T], BF16, kind="ExternalInput").ap()
    # host-packed partition-major weights, flat per-partition rows so
    # every weight DMA lowers to a plain 2D descriptor
    wqk0 = nc.dram_tensor("wqk0", [128, 2 * NCT * 128], BF16,
                          kind="ExternalInput").ap()
    wqk12 = nc.dram_tensor("wqk12", [128, 2 * 2 * NCT * 128], BF16,
                           kind="ExternalInput").ap()
    wv = nc.dram_tensor("wv", [128, NCT * HL * HS], BF16,
                        kind="ExternalInput").ap()
    wpt = nc.dram_tensor("wpt", [128, 3 * C], BF16,
                         kind="ExternalInput").ap()
    bpb = nc.dram_tensor("bpb", [1, C], F32, kind="ExternalInput").ap()
    y = nc.dram_tensor("y", [T, C], BF16, kind="ExternalOutput").ap()

    with tile.TileContext(nc) as tc:
        with (
            tc.tile_pool(name="consts", bufs=1) as consts,
            tc.tile_pool(name="xw", bufs=1) as xw,
            tc.tile_pool(name="pt", bufs=21) as ptp,
            tc.tile_pool(name="small", bufs=6) as small,
            tc.tile_pool(name="ysb", bufs=6) as ysbp,
            # PSUM budget (8 banks): st 2x2 + otu 1x2 + tt 2x1
            tc.tile_pool(name="ps_st", bufs=2, space="PSUM") as ps_st,
            tc.tile_pool(name="ps_otu", bufs=1, space="PSUM") as ps_otu,
            tc.tile_pool(name="ps_t", bufs=2, space="PSUM") as ps_t,
        ):
            # ---------------- input DMAs ----------------
            # scalar queue: pair-0 QK weights (one DMA), then odd x halves.
            # sync queue: even x halves, then V/proj weights and the rest.
            # First-QK needs wqk[:, 0] + the h0 halves of every band.
            wqk0_sb = xw.tile([128, 2, NCT, 128], BF16, tag="wqk0",
                              name="wqk0")
            nc.scalar.dma_start(
                out=wqk0_sb.rearrange("p a b c -> p (a b c)"), in_=wqk0)
            xt = [xw.tile([128, T], BF16, tag=f"xt{ci}", name=f"xt{ci}")
                  for ci in range(NCT)]
            for h in range(2):
                for ci in range(NCT):
                    q = nc.sync if ci % 2 == 0 else nc.scalar
                    q.dma_start(
                        out=xt[ci][:, h * 1024:(h + 1) * 1024],
                        in_=xT[ci * 128:(ci + 1) * 128,
                               h * 1024:(h + 1) * 1024],
                    )
            wv_sb = xw.tile([128, NCT, HL * HS], BF16, tag="wv", name="wv")
            nc.sync.dma_start(
                out=wv_sb.rearrange("p a b -> p (a b)"), in_=wv)
            wqk12_sb = xw.tile([128, 2, 2, NCT, 128], BF16, tag="wqk12",
                               name="wqk12")
            nc.sync.dma_start(
                out=wqk12_sb.rearrange("p a b c d -> p (a b c d)"),
                in_=wqk12)
            wpt_sb = consts.tile([128, 3, C], BF16, tag="wpt", name="wpt")
            nc.sync.dma_start(
                out=wpt_sb.rearrange("p a b -> p (a b)"), in_=wpt)
            bprow = consts.tile([1, C], F32, tag="bprow", name="bprow")
            nc.scalar.dma_start(out=bprow, in_=bpb)

            # persistent tensors; augmented V column 0 carries the ones row
            # so the softmax denominator lands at psum partition 0.
            vaug = consts.tile([128, NTT, HL, HS + 1], BF16)
            nc.gpsimd.memset(vaug[:, :, :, HS:HS + 1], 1.0)
            pairQ = [consts.tile([128, T], BF16, tag=f"pq{p}", name=f"pq{p}")
                     for p in range(3)]
            pairK = [consts.tile([128, T], BF16, tag=f"pk{p}", name=f"pk{p}")
                     for p in range(3)]
            otn = consts.tile([128, 3, T], BF16)
            ones_rows = consts.tile([128, HS + 1], BF16)
            nc.gpsimd.memset(ones_rows, 1.0)
            # upper-triangular (keep c >= r) mask for the diagonal subtiles
            trimask = consts.tile([128, 2, 128], BF16)
            nc.gpsimd.memset(trimask, 1.0)
            nc.gpsimd.affine_select(
                out=trimask, in_=trimask,
                compare_op=mybir.AluOpType.is_ge,
                fill=0.0, base=0,
                pattern=[[0, 2], [1, 128]],
                channel_multiplier=-1,
            )
            # bias broadcast to all 128 partitions (off the critical path)
            bias128 = consts.tile([128, C], F32)
            nc.gpsimd.partition_broadcast(bias128, bprow, channels=128)

            import contextlib
            rep_ctx = (
                tc.For_i(0, repeat, 1,
                         hint_engines=(mybir.EngineType.PE,
                                       mybir.EngineType.DVE,
                                       mybir.EngineType.Activation,
                                       mybir.EngineType.SP,
                                       mybir.EngineType.Pool))
                if repeat > 1 else contextlib.nullcontext()
            )
            with rep_ctx:
                build_phases(nc, tc, consts, xw, ptp, small, ysbp,
                             ps_st, ps_otu, ps_t,
                             xt, wqk0_sb, wqk12_sb, wv_sb, wpt_sb,
                             vaug, pairQ, pairK, otn, ones_rows,
                             trimask, bias128, y)

    nc.compile()
    return nc


def build_phases(nc, tc, consts, xw, ptp, small, ysbp,
                 ps_st, ps_otu, ps_t,
                 xt, wqk0_sb, wqk12_sb, wv_sb, wpt_sb,
                 vaug, pairQ, pairK, otn, ones_rows, trimask, bias128, y):
    def wqk_ap(p, which, ci):
        if p == 0:
            return wqk0_sb[:, which, ci, :]
        return wqk12_sb[:, p - 1, which, ci, :]

    # ---- filler unit builders (each issues one psum-group of PE work) ----
    def qk_unit(p, which, m):
        """Project one 512-col chunk of Q (which=0) or K (which=1) for pair p
        straight into pairQ/pairK (partition-aligned, no staging DMA)."""
        dst = pairQ[p] if which == 0 else pairK[p]
        sl = slice(m * 512, (m + 1) * 512)
        ps = ps_t.tile([128, 512], F32, tag="tt", name=f"qk{p}_{which}_{m}")
        for ci in range(NCT):
            nc.tensor.matmul(ps, wqk_ap(p, which, ci), xt[ci][:, sl],
                             start=(ci == 0), stop=(ci == NCT - 1))
        nc.vector.tensor_copy(out=dst[:, sl], in_=ps)

    def v_unit(tt):
        ps = ps_t.tile([128, HL * HS], F32, tag="tt", name=f"psv{tt}")
        for ci in range(NCT):
            nc.tensor.matmul(ps, xt[ci][:, tt * 128:(tt + 1) * 128],
                             wv_sb[:, ci, :],
                             start=(ci == 0), stop=(ci == NCT - 1))
        nc.vector.tensor_copy(
            out=vaug[:, tt, :, 0:HS],
            in_=ps.rearrange("p (h d) -> p h d", h=HL),
        )

    def norm_rb(state):
        """Row-broadcast of the reciprocal rows on GPSIMD.  The ucode reads
        literal partition 0, so hop the row down first with a tiny DMA."""
        p, m, otu_sb, otu_ps, rbs = state
        for e in range(2):
            stg = small.tile([1, 512], F32, tag="rstg", name=f"rs{p}_{m}_{e}")
            nc.sync.dma_start(out=stg, in_=otu_sb[HS:HS + 1, e, :])
            rb = small.tile([HS, 512], F32, tag="rbb", name=f"rb{p}_{m}_{e}")
            nc.gpsimd.partition_broadcast(rb, stg, channels=HS)
            rbs.append(rb)

    def norm_mul(state, from_psum=False):
        p, m, otu_sb, otu_ps, rbs = state
        for e in range(2):
            otnorm = small.tile([HS, 512], BF16, tag="otnorm", name="otnorm")
            body = (otu_ps if from_psum else otu_sb)[0:HS, e, :]
            nc.vector.tensor_mul(out=otnorm, in0=body, in1=rbs[e])
            nc.sync.dma_start(
                out=otn[64 * e:64 * e + HS, p, m * 512:(m + 1) * 512],
                in_=otnorm,
            )

    def proj_unit(tt, tail=False):
        y1 = ps_t.tile([128, 512], F32, tag="tt", name=f"y1_{tt}")
        y2 = ps_t.tile([128, 256], F32, tag="tt", name=f"y2_{tt}")
        for g in range(3):
            lhs = otn[:, g, tt * 128:(tt + 1) * 128]
            nc.tensor.matmul(y1, lhs, wpt_sb[:, g, 0:512],
                             start=(g == 0), stop=(g == 2))
            nc.tensor.matmul(y2, lhs, wpt_sb[:, g, 512:768],
                             start=(g == 0), stop=(g == 2))
        ysb = ysbp.tile([128, C], BF16, tag="ysb", name="ysb")
        # psum->sbuf copy fused with the device-side bias add; half-granular
        # so the store overlaps the second half.
        nc.vector.tensor_add(out=ysb[:, 0:512], in0=y1, in1=bias128[:, 0:512])
        nc.sync.dma_start(out=y[tt * 128:(tt + 1) * 128, 0:512],
                          in_=ysb[:, 0:512])
        nc.vector.tensor_add(out=ysb[:, 512:768], in0=y2,
                             in1=bias128[:, 512:768])
        q = nc.scalar if tail else nc.sync
        q.dma_start(out=y[tt * 128:(tt + 1) * 128, 512:768],
                    in_=ysb[:, 512:768])

    # ---- static filler schedule ----
    fillers = {
        (0, 0): [lambda tt=t: v_unit(tt) for t in range(0, 4)]
                + [lambda: qk_unit(0, 0, 1), lambda: qk_unit(0, 1, 1)],
        (0, 1): [lambda tt=t: v_unit(tt) for t in range(4, 8)]
                + [lambda: qk_unit(0, 0, 2), lambda: qk_unit(0, 1, 2)],
        (0, 2): [lambda tt=t: v_unit(tt) for t in range(8, 12)]
                + [lambda: qk_unit(0, 0, 3), lambda: qk_unit(0, 1, 3)],
        (0, 3): [lambda tt=t: v_unit(tt) for t in range(12, 16)]
                + [lambda: qk_unit(1, 0, 0), lambda: qk_unit(1, 1, 0),
                   lambda: qk_unit(1, 0, 1), lambda: qk_unit(1, 1, 1)],
        (1, 0): [lambda: qk_unit(1, 0, 2), lambda: qk_unit(1, 1, 2)],
        (1, 1): [lambda: qk_unit(1, 0, 3), lambda: qk_unit(1, 1, 3),
                 lambda: qk_unit(2, 0, 0), lambda: qk_unit(2, 1, 0)],
        (1, 2): [lambda: qk_unit(2, 0, 1), lambda: qk_unit(2, 1, 1),
                 lambda: qk_unit(2, 0, 2), lambda: qk_unit(2, 1, 2)],
        (1, 3): [],
        (2, 0): [lambda: qk_unit(2, 0, 3), lambda: qk_unit(2, 1, 3)],
        (2, 1): [lambda tt=t: proj_unit(tt) for t in range(0, 4)],
        (2, 2): [lambda tt=t: proj_unit(tt) for t in range(4, 8)],
        (2, 3): [lambda tt=t: proj_unit(tt) for t in range(8, 12)],
    }

    # HAM warmup: dependency-free dummy matmuls stream while the input DMAs
    # land, so the PE clock gate is at 8/8 (2.4 GHz) when real work starts.
    wu = ps_t.tile([HS + 1, 64], F32, tag="tt", name="warmup")
    for _ in range(28):
        nc.tensor.matmul(wu, ones_rows, ones_rows[:, 0:64],
                         start=True, stop=True)

    # prefix: only pair-0 chunk-0 Q,K before attention starts
    qk_unit(0, 0, 0)
    qk_unit(0, 1, 0)

    pending_norm = []  # deferred normalization units

    for p in range(3):
        for m in range(NTC):
            jmax = 4 * m + 3
            last_chunk = (p == 2 and m == NTC - 1)
            fl = list(fillers[(p, m)])
            fl = pending_norm + fl
            pending_norm = []
            fi = 0

            PV_LAG = 5 if jmax >= 5 else (jmax if jmax >= 1 else 1)

            def pv(jj):
                ss = max(0, jj - 4 * m)
                for e in range(2):
                    nc.tensor.matmul(
                        otu_ps[:, e, 128 * ss:512],
                        vaug[:, jj, 2 * p + e, :],
                        pts[jj][:, e, 128 * ss:512],
                        start=(jj == 0), stop=(jj == jmax),
                        skip_group_check=True,
                    )

            otu_ps = ps_otu.tile([HS + 1, 2, 512], F32, tag="otu", name="otu")
            pts = []
            for j in range(jmax + 1):
                s0 = max(0, j - 4 * m)
                st = ps_st.tile([128, 2, 512], F32, tag="st", name="st")
                for e in range(2):
                    nc.tensor.matmul(
                        st[:, e, 128 * s0:512],
                        pairK[p][64 * e:64 * e + 64, j * 128:(j + 1) * 128],
                        pairQ[p][64 * e:64 * e + 64,
                                 m * 512 + 128 * s0:(m + 1) * 512],
                        start=True, stop=True,
                        tile_position=(64 * e, 0),
                    )
                pt = ptp.tile([128, 2, 512], BF16, tag="pt", name="pt")
                pts.append(pt)
                nc.scalar.activation(
                    out=pt[:, :, 128 * s0:512],
                    in_=st[:, :, 128 * s0:512],
                    func=mybir.ActivationFunctionType.Exp,
                    scale=SCALE,
                )
                if j >= 4 * m:
                    # zero below-diagonal of the diagonal subtile (both
                    # heads) with a triangular-mask multiply on DVE.
                    nc.vector.tensor_mul(
                        out=pt[:, :, 128 * s0:128 * s0 + 128],
                        in0=pt[:, :, 128 * s0:128 * s0 + 128],
                        in1=trimask,
                    )
                # pace fillers evenly across the chunk
                while fi < len(fl) and (j + 1) * len(fl) >= (fi + 1) * (jmax + 1):
                    fl[fi]()
                    fi += 1
                # PV lags the exp by PV_LAG j-steps for pipeline slack
                if j >= PV_LAG:
                    pv(j - PV_LAG)
            # drain leftover fillers, then the last PVs
            while fi < len(fl):
                fl[fi]()
                fi += 1
            for jj in range(max(0, jmax + 1 - PV_LAG), jmax + 1):
                pv(jj)
            # reciprocal straight from PSUM at partition 0, then stash the
            # body to SBUF to free the psum for the next chunk; rb/mul are
            # deferred into the next chunk's filler slots.  For the last
            # chunk run the chain immediately and multiply straight out of
            # the psum.
            otu_sb = small.tile([HS + 1, 2, 512], F32, tag="otusb",
                                name="otusb")
            for e in range(2):
                nc.vector.reciprocal(out=otu_sb[HS:HS + 1, e, :],
                                     in_=otu_ps[HS:HS + 1, e, :])
            if last_chunk:
                state = (p, m, otu_sb, otu_ps, [])
                norm_rb(state)
                norm_mul(state, from_psum=True)
            else:
                nc.vector.tensor_copy(out=otu_sb[0:HS], in_=otu_ps[0:HS])
                state = (p, m, otu_sb, None, [])
                pending_norm = [lambda s=state: norm_rb(s),
                                lambda s=state: norm_mul(s)]

    # tail: final projection row
    for tt in range(12, 16):
        proj_unit(tt, tail=True)


_NC_CACHE = {}


def get_nc(repeat=1):
    key = repeat
    if key not in _NC_CACHE:
        nc = bacc.Bacc(
            "TRN2", target_bir_lowering=False, debug=False, num_devices=8
        )
        _NC_CACHE[key] = build_kernel(nc, repeat=repeat)
    return _NC_CACHE[key]


def make_in_maps(x, Wq, Wk, Wv, Wp, bp=None):
    x = np.asarray(x, dtype=np.float32)
    Wq = np.asarray(Wq, dtype=np.float32)
    Wk = np.asarray(Wk, dtype=np.float32)
    Wv = np.asarray(Wv, dtype=np.float32)
    Wp = np.asarray(Wp, dtype=np.float32)
    if bp is None:
        bp = np.zeros((C,), dtype=np.float32)
    bp = np.asarray(bp, dtype=np.float32)
    bf = ml_dtypes.bfloat16
    # the two cores of a batch pair share xT; the four cores of a head
    # parity share all weights -- build each distinct block once
    xTs = {b: np.ascontiguousarray(x[b].T).astype(bf) for b in range(B)}
    wsets = {}
    for par in range(2):
        hs = HL * par
        # wqk[r, p, which, ci, col] = W_{q/k}[head pair p stacked][ci*128+r, col]
        wqk_ = np.empty((128, 3, 2, NCT, 128), dtype=bf)
        for p in range(3):
            sq = np.concatenate([Wq[hs + 2 * p], Wq[hs + 2 * p + 1]], axis=1)
            sk = np.concatenate([Wk[hs + 2 * p], Wk[hs + 2 * p + 1]], axis=1)
            wqk_[:, p, 0] = sq.reshape(NCT, 128, 128).transpose(1, 0, 2)
            wqk_[:, p, 1] = sk.reshape(NCT, 128, 128).transpose(1, 0, 2)
        wv_full = np.transpose(Wv[hs:hs + HL], (1, 0, 2)).reshape(C, HL * HS)
        wv_ = np.ascontiguousarray(
            wv_full.reshape(NCT, 128, HL * HS).transpose(1, 0, 2)
        ).astype(bf)
        wpt_ = np.ascontiguousarray(
            Wp[:, hs * HS:(hs + HL) * HS].T.reshape(3, 128, C).transpose(1, 0, 2)
        ).astype(bf)
        wsets[par] = {
            "wqk0": np.ascontiguousarray(wqk_[:, 0].reshape(128, -1)),
            "wqk12": np.ascontiguousarray(wqk_[:, 1:3].reshape(128, -1)),
            "wv": wv_.reshape(128, -1),
            "wpt": wpt_.reshape(128, -1),
        }
    bpb0 = bp.reshape(1, C).astype(np.float32)
    bpb1 = np.zeros((1, C), dtype=np.float32)
    in_maps = []
    for c in range(8):
        in_maps.append({"xT": xTs[c // 2],
                        **wsets[c % 2],
                        "bpb": bpb0 if c % 2 == 0 else bpb1})
    return in_maps


def run(x, Wq, Wk, Wv, Wp, bp, trace=False):
    nc = get_nc()
    in_maps = make_in_maps(x, Wq, Wk, Wv, Wp, bp)
    res = bass_utils.run_bass_kernel_spmd(
        nc, in_maps, core_ids=list(range(8)), trace=trace
    )
    y = np.zeros((B, T, C), dtype=np.float32)
    for c in range(8):
        y[c // 2] += np.asarray(res.results[c]["y"], dtype=np.float32)
    return y, res


def kernel(x, Wq, Wk, Wv, Wp, bp):
    y, _ = run(x, Wq, Wk, Wv, Wp, bp)
    return y


def make_runner(nc):
    """Build the sharded PJRT callable once. Returns (fn, prep, zeros,
    out_names, make_loop_fn)."""
    import jax
    from jax.experimental.shard_map import shard_map
    from jax.sharding import Mesh, PartitionSpec, NamedSharding
    from concourse import mybir as _mybir
    from concourse.bass2jax import (
        _bass_exec_p, install_neuronx_cc_hook, partition_id_tensor,
    )

    install_neuronx_cc_hook()
    n_cores = 8
    partition_name = (
        nc.partition_id_tensor.name if nc.partition_id_tensor else None
    )
    in_names, out_names, out_avals = [], [], []
    for alloc in nc.m.functions[0].allocations:
        if not isinstance(alloc, _mybir.MemoryLocationSet):
            continue
        name = alloc.memorylocations[0].name
        if alloc.kind == "ExternalInput":
            if name != partition_name:
                in_names.append(name)
        elif alloc.kind == "ExternalOutput":
            out_names.append(name)
            out_avals.append(
                jax.core.ShapedArray(
                    tuple(alloc.tensor_shape), _mybir.dt.np(alloc.dtype)
                )
            )
    n_params = len(in_names)
    n_outs = len(out_avals)
    all_in_names = in_names + out_names
    if partition_name is not None:
        all_in_names.append(partition_name)

    def _body(*args):
        operands = list(args)
        if partition_name is not None:
            operands.append(partition_id_tensor())
        outs = _bass_exec_p.bind(
            *operands,
            out_avals=tuple(out_avals),
            in_names=tuple(all_in_names),
            out_names=tuple(out_names),
            lowering_input_output_aliases=(),
            sim_require_finite=True,
            sim_require_nnan=True,
            nc=nc,
        )
        return tuple(outs)

    devices = jax.devices()[:n_cores]
    mesh = Mesh(np.array(devices), ("core",))
    sharded = jax.jit(
        shard_map(
            _body, mesh=mesh,
            in_specs=(PartitionSpec("core"),) * (n_params + n_outs),
            out_specs=(PartitionSpec("core"),) * n_outs,
            check_rep=False,
        ),
        donate_argnums=tuple(range(n_params, n_params + n_outs)),
        keep_unused=True,
    )
    shd = NamedSharding(mesh, PartitionSpec("core"))

    def prep(in_maps):
        return [
            jax.device_put(
                np.concatenate([in_maps[c][nm] for c in range(n_cores)], axis=0),
                shd,
            )
            for nm in in_names
        ]

    def zeros():
        dz = [
            jax.device_put(
                np.zeros((n_cores * a.shape[0], *a.shape[1:]), a.dtype), shd
            )
            for a in out_avals
        ]
        # the donated output buffers are setup, not execution: finish the
        # host->device transfer before the caller starts its timer
        jax.block_until_ready(dz)
        return dz

    def fn(dev_inputs, dev_zeros):
        outs = sharded(*dev_inputs, *dev_zeros)
        jax.block_until_ready(outs)
        return outs

    def make_loop_fn(n_iters):
        def _body_n(*args):
            ins = args[:n_params]
            carry = tuple(args[n_params:])

            def step(i, carry):
                operands = list(ins) + list(carry)
                if partition_name is not None:
                    operands.append(partition_id_tensor())
                outs = _bass_exec_p.bind(
                    *operands,
                    out_avals=tuple(out_avals),
                    in_names=tuple(all_in_names),
                    out_names=tuple(out_names),
                    lowering_input_output_aliases=(),
                    sim_require_finite=True,
                    sim_require_nnan=True,
                    nc=nc,
                )
                return tuple(outs)

            return jax.lax.fori_loop(0, n_iters, step, carry)

        looped = jax.jit(
            shard_map(
                _body_n, mesh=mesh,
                in_specs=(PartitionSpec("core"),) * (n_params + n_outs),
                out_specs=(PartitionSpec("core"),) * n_outs,
                check_rep=False,
            ),
            donate_argnums=tuple(range(n_params, n_params + n_outs)),
            keep_unused=True,
        )

        def run_n(dev_inputs, dev_zeros):
            outs = looped(*dev_inputs, *dev_zeros)
            jax.block_until_ready(outs)
            return outs

        return run_n

    return fn, prep, zeros, out_names, make_loop_fn
